# revision 22
# baseline (speedup 1.0000x reference)
# Trainium2 Bass kernel for nn_LiquidMalwareDetector.
#
# Strategy: pure data parallelism over the batch dim (8192 -> 1024 per core,
# 8 cores). Host side only reshapes/shards (layout, no arithmetic); all math
# runs on device:
#   phase 1: per-core BN channel sums/sumsq (matmul-with-0/1-matrix reduction)
#            + 8-core AllReduce of the 6 partial stats
#   phase 2b: fit a degree-10 Chebyshev polynomial (in clipped time c) to the
#            128 gate curves g_h(c)=sigmoid((A_h c + B_h) c + C_h) ON DEVICE:
#            sample sigmoid at 512 Chebyshev nodes (PE matmul + ACT sigmoid),
#            then project onto the basis with a constant pinv matrix (PE).
#            This removes the sigmoid ACT op from the scan entirely - the
#            per-step gate becomes a K=11 PE matmul against the coefficients.
#   phase 2: fold BN affine into the x-columns of the ff weights + bias
#   phase 3: stage the bf16 Chebyshev basis rows T_0..T_10(c') for every step
#            to DRAM (DVE recurrence over 128-step tiles; overlaps the scan)
#   phase 4: 1024-step recurrence in feature-major layout, two independent
#            512-column batch chains interleaved to hide the serial latency.
#            State is the unsummed blend pair rz=[g*t1; (1-g)*t2]; duplicated
#            Wh rows in the K=128 matmul sum the pair. Per-iteration PE issue
#            order is [gateA', xA', mainA, gateB', xB', mainB] so ready work
#            sits ahead of each blocking main matmul. The x-contribution for
#            step s+1 is pre-accumulated into the next (2-bank wide) PSUM
#            tile so tanh starts right after the main matmul drains; gate and
#            x run as single N=1024 bf16 streams on disjoint PE row groups
#            (q0/q64), concurrently. The blend multiplies tanh output (SBUF
#            f16) with the gate (PSUM f32) in one DVE op.
#   phase 5: classifier via sigmoid of logit-difference (== 2-class softmax)
import numpy as np
import ml_dtypes

NUM_CORES = 8
B_FULL = 8192
S_FULL = 1024
F = 3
H = 64
BN_EPS = 1e-5
DEG = 10            # Chebyshev degree for the gate fit
NK = DEG + 1        # basis rows
MFIT = 512          # sampling nodes for the on-device fit

_CACHE = {}


def _build(num_cores, s_steps, b_loc):
    from concourse import bacc, mybir
    import concourse.tile as tile

    f16 = mybir.dt.float16
    bf16 = mybir.dt.bfloat16
    f32 = mybir.dt.float32
    Alu = mybir.AluOpType
    Act = mybir.ActivationFunctionType

    NG = b_loc // 512  # 512-column groups per step
    assert b_loc % 512 == 0

    nc = bacc.Bacc(
        "TRN2",
        target_bir_lowering=False,
        debug=False,
        num_devices=num_cores,
    )

    # ---- I/O -------------------------------------------------------------
    xT = nc.dram_tensor("xT", [s_steps * F, b_loc], bf16,
                        kind="ExternalInput")
    tT = nc.dram_tensor("tT", [s_steps, b_loc], f32, kind="ExternalInput")
    # wzdup: [128,128] duplicated h-weights (rows 0-63 == rows 64-127 == Wh.T)
    # so the matmul itself sums the unsummed blend halves [u1; u2].
    wzdup_d = nc.dram_tensor("wzdup", [2 * H, 2 * H], f16, kind="ExternalInput")
    wzx_d = nc.dram_tensor("wzx", [F, 2 * H], f16, kind="ExternalInput")
    wzxb_d = nc.dram_tensor("wzxb", [F, 2 * H], bf16, kind="ExternalInput")
    idup_d = nc.dram_tensor("idup", [2 * H, H], f16, kind="ExternalInput")
    bz_d = nc.dram_tensor("bz", [2 * H, 1], f32, kind="ExternalInput")
    # gate fit inputs: wg3 rows = [A;B;C] per h-pair column (+/- stacked),
    # gridpow rows = [c^2; c; 1] at the sampling nodes, wfit44 = the constant
    # LSQ projection matrix chunked [128, 4*11].
    wg3_d = nc.dram_tensor("wg3", [3, 2 * H], f16, kind="ExternalInput")
    gridpow_d = nc.dram_tensor("gridpow", [3, MFIT], f16, kind="ExternalInput")
    wfit_d = nc.dram_tensor("wfit44", [MFIT // 4, 4 * NK], f16,
                            kind="ExternalInput")
    eT_d = nc.dram_tensor("eT", [128, 9], bf16, kind="ExternalInput")
    gam_d = nc.dram_tensor("gam", [F, 1], f32, kind="ExternalInput")
    bet_d = nc.dram_tensor("bet", [F, 1], f32, kind="ExternalInput")
    wcls_d = nc.dram_tensor("wcls", [H, 2], f16, kind="ExternalInput")
    bcls_d = nc.dram_tensor("bcls", [2, 1], f32, kind="ExternalInput")
    probs_d = nc.dram_tensor("probs", [2, b_loc], f32, kind="ExternalOutput")

    inv_bs = 1.0 / float(num_cores * b_loc * s_steps)

    with tile.TileContext(nc) as tc:
        with tc.tile_pool(name="const", bufs=1) as cpool, \
             tc.tile_pool(name="dram", bufs=1, space="DRAM") as dpool:
            # persistent SBUF-resident weights/constants
            wzdup = cpool.tile([2 * H, 2 * H], f16)
            nc.sync.dma_start(wzdup[:], wzdup_d[:])
            wzx = cpool.tile([F, 2 * H], f16)
            nc.sync.dma_start(wzx[:], wzx_d[:])
            # folded x-weights, placed at partitions 64..66 so the x matmul's
            # lhsT/rhs base partitions match (row group q64, disjoint from the
            # gate matmul's q0 rows -> the two run concurrently on the PE)
            wzx6 = cpool.tile([96 + F, 2 * H], bf16)
            idup = cpool.tile([2 * H, H], f16)
            nc.sync.dma_start(idup[:], idup_d[:])
            bz = cpool.tile([2 * H, 1], f32)
            nc.sync.dma_start(bz[:], bz_d[:])
            wg3 = cpool.tile([3, 2 * H], f16)
            nc.sync.dma_start(wg3[:], wg3_d[:])
            gridpow = cpool.tile([3, MFIT], f16)
            nc.sync.dma_start(gridpow[:], gridpow_d[:])
            wfit = cpool.tile([MFIT // 4, 4 * NK], f16)
            nc.sync.dma_start(wfit[:], wfit_d[:])
            eT = cpool.tile([128, 9], bf16)
            nc.sync.dma_start(eT[:], eT_d[:])
            gam = cpool.tile([F, 1], f32)
            nc.sync.dma_start(gam[:], gam_d[:])
            bet = cpool.tile([F, 1], f32)
            nc.sync.dma_start(bet[:], bet_d[:])
            wcls = cpool.tile([H, 2], f16)
            nc.sync.dma_start(wcls[:], wcls_d[:])
            bcls = cpool.tile([2, 1], f32)
            nc.sync.dma_start(bcls[:], bcls_d[:])
            zb_f = cpool.tile([2 * H, 1], f32)   # folded tanh bias
            stats_g = cpool.tile([F, 2], f32)    # all-reduced [sum, sumsq]
            gcoef = cpool.tile([32 + NK, 2 * H], bf16)  # gate coefficients
            # (rows 0-10 serve chain A at row group q0, a DMA-duplicated copy
            # at rows 32-42 serves chain B at q32 so all four per-step stream
            # matmuls land on distinct PE row groups and run concurrently)
            ones = cpool.tile([128, b_loc], bf16)  # T_0 plane
            nc.vector.memset(ones[:], 1.0)

            # ---- phase 3: Chebyshev basis staging ------------------------
            # chd row s holds the A-half block [T_0..T_10] then the B-half
            # block, each 512 wide, so the scan reads two contiguous 11KB
            # blocks per step. Tile 0 (steps 0-127) is built before the scan;
            # tiles 1..7 are interleaved into the scan loop one DVE op per
            # step so the scan's blends are not queued behind ~110us of
            # serial basis work on the Vector engine.
            chd = dpool.tile([s_steps, 2 * NK * 512], bf16)
            cgp = tc.alloc_tile_pool(name="cg", bufs=2)
            cgk = tc.alloc_tile_pool(name="cgk", bufs=4)

            def ph3_tile(i):
                r0 = i * 128
                nr = min(128, s_steps - r0)
                tt = cgp.tile([128, b_loc], f32, tag="tt", name="tt")
                nc.gpsimd.dma_start(tt[0:nr, :], tT[r0:r0 + nr, :])
                cc = cgp.tile([128, b_loc], f32, tag="cc", name="cc")
                nc.vector.tensor_scalar(cc[0:nr, :], tt[0:nr, :], 60.0,
                                        0.0, Alu.min, Alu.max)
                nc.gpsimd.dma_start(chd[r0:r0 + nr, 0:512],
                                    ones[0:nr, 0:512])
                nc.gpsimd.dma_start(
                    chd[r0:r0 + nr, NK * 512:NK * 512 + 512],
                    ones[0:nr, 512:b_loc])
                yield
                t1c = cgk.tile([128, b_loc], bf16, tag="tk", name="t1c")
                nc.vector.tensor_scalar(t1c[0:nr, :], cc[0:nr, :], 0.2,
                                        -1.0, Alu.mult, Alu.add)
                nc.gpsimd.dma_start(chd[r0:r0 + nr, 512:1024],
                                    t1c[0:nr, 0:512])
                nc.gpsimd.dma_start(
                    chd[r0:r0 + nr, (NK + 1) * 512:(NK + 2) * 512],
                    t1c[0:nr, 512:b_loc])
                yield
                dd = cgp.tile([128, b_loc], bf16, tag="dd", name="dd")
                nc.vector.tensor_scalar(dd[0:nr, :], cc[0:nr, :], 0.4,
                                        -2.0, Alu.mult, Alu.add)
                yield
                prev2, prev1 = ones, t1c
                for k in range(2, NK):
                    tmp = cgp.tile([128, b_loc], bf16, tag="tmp", name="tmp")
                    nc.vector.tensor_tensor(tmp[0:nr, :], dd[0:nr, :],
                                            prev1[0:nr, :], Alu.mult)
                    yield
                    tk = cgk.tile([128, b_loc], bf16, tag="tk", name="tk")
                    nc.vector.tensor_tensor(tk[0:nr, :], tmp[0:nr, :],
                                            prev2[0:nr, :], Alu.subtract)
                    nc.gpsimd.dma_start(
                        chd[r0:r0 + nr, k * 512:(k + 1) * 512],
                        tk[0:nr, 0:512])
                    nc.gpsimd.dma_start(
                        chd[r0:r0 + nr, (NK + k) * 512:(NK + k + 1) * 512],
                        tk[0:nr, 512:b_loc])
                    yield
                    prev2, prev1 = prev1, tk

            import itertools
            ph3_rest = itertools.chain(
                *[ph3_tile(i) for i in range(1, (s_steps + 127) // 128)])

            # ---- phase 1: BN stats ---------------------------------------
            rows_total = s_steps * F
            with tc.tile_pool(name="st_ps", bufs=1, space="PSUM") as stps, \
                 tc.tile_pool(name="st_sb", bufs=3) as stsb:
                psum_s = stps.tile([F, b_loc], f32)
                psum_q = stps.tile([F, b_loc], f32)
                n_tiles = (rows_total + 127) // 128
                for i in range(n_tiles):
                    r0 = i * 128
                    nr = min(128, rows_total - r0)
                    ph = r0 % 3
                    xst = stsb.tile([128, b_loc], bf16, tag="xst")
                    nc.sync.dma_start(xst[0:nr, :], xT[r0:r0 + nr, :])
                    xsq = stsb.tile([128, b_loc], bf16, tag="xsq")
                    nc.vector.tensor_tensor(
                        xsq[0:nr, :], xst[0:nr, :], xst[0:nr, :], Alu.mult)
                    first = i == 0
                    last = i == n_tiles - 1
                    for g in range(NG):
                        cs = slice(g * 512, (g + 1) * 512)
                        nc.tensor.matmul(
                            psum_s[:, cs], eT[0:nr, 3 * ph:3 * ph + 3],
                            xst[0:nr, cs], start=first, stop=last)
                        nc.tensor.matmul(
                            psum_q[:, cs], eT[0:nr, 3 * ph:3 * ph + 3],
                            xsq[0:nr, cs], start=first, stop=last)
                stats_l = stsb.tile([F, 2], f32, tag="stl")
                nc.vector.tensor_reduce(
                    stats_l[:, 0:1], psum_s[:], mybir.AxisListType.X, Alu.add)
                nc.vector.tensor_reduce(
                    stats_l[:, 1:2], psum_q[:], mybir.AxisListType.X, Alu.add)
                # 6-float AllReduce across the 8 cores via DRAM bounce
                cc_in = dpool.tile([F, 2], f32)
                cc_out = dpool.tile([F, 2], f32, addr_space="Shared")
                nc.sync.dma_start(cc_in[:], stats_l[:])
                nc.gpsimd.collective_compute(
                    "AllReduce", Alu.add,
                    replica_groups=[list(range(num_cores))],
                    ins=[cc_in.opt()], outs=[cc_out.opt()])
                # the 6-float AllReduce takes ~18us of wall clock with no
                # engine work; build Chebyshev tile 0 (which gates the scan
                # start) on the Vector engine inside that window
                for _ in ph3_tile(0):
                    pass
                nc.sync.dma_start(stats_g[:], cc_out[:])

            # ---- phase 2b: on-device gate fit + HAM bridge + BN fold -----
            with tc.tile_pool(name="fold", bufs=1) as fp, \
                 tc.tile_pool(name="gfit", bufs=2) as gf, \
                 tc.tile_pool(name="fold_ps", bufs=2, space="PSUM") as fps, \
                 tc.tile_pool(name="coef_ps", bufs=1, space="PSUM") as cps:
                # sample the 128 gate sigmoid curves at MFIT Chebyshev nodes
                # and project onto the degree-DEG basis (all on PE/ACT).
                pcoef = cps.tile([NK, 2 * H], f32)
                for m in range(4):
                    pgs = fps.tile([128, 128], f32, tag="pgs")
                    nc.tensor.matmul(
                        pgs[:], gridpow[:, m * 128:(m + 1) * 128], wg3[:],
                        start=True, stop=True)
                    gsm = gf.tile([128, 128], f16, tag="gsm")
                    nc.scalar.activation(gsm[:], pgs[:], Act.Sigmoid)
                    nc.tensor.matmul(
                        pcoef[:], wfit[:, m * NK:(m + 1) * NK], gsm[:],
                        start=(m == 0), stop=(m == 3))
                nc.vector.tensor_copy(gcoef[0:NK, :], pcoef[:])
                nc.sync.dma_start(gcoef[32:32 + NK, :], gcoef[0:NK, :])

                # ---- phase 2: BN fold ------------------------------------
                mean = fp.tile([F, 1], f32)
                nc.vector.tensor_scalar(mean[:], stats_g[:, 0:1], inv_bs, None,
                                        Alu.mult)
                msq = fp.tile([F, 1], f32)
                nc.vector.tensor_scalar(msq[:], stats_g[:, 1:2], inv_bs, None,
                                        Alu.mult)
                var = fp.tile([F, 1], f32)
                nc.vector.tensor_tensor(var[:], mean[:], mean[:], Alu.mult)
                nc.vector.tensor_tensor(var[:], msq[:], var[:], Alu.subtract)
                veps = fp.tile([F, 1], f32)
                nc.vector.tensor_scalar(veps[:], var[:], BN_EPS, None, Alu.add)
                # rsqrt: ACT sqrt seed + DVE reciprocal + 2 Newton iters
                sq = fp.tile([F, 1], f32)
                nc.scalar.activation(sq[:], veps[:], Act.Sqrt)
                y = fp.tile([F, 1], f32)
                nc.vector.reciprocal(y[:], sq[:])
                t1 = fp.tile([F, 1], f32)
                t2 = fp.tile([F, 1], f32)
                for _ in range(2):
                    nc.vector.tensor_tensor(t1[:], y[:], y[:], Alu.mult)
                    nc.vector.tensor_tensor(t2[:], t1[:], veps[:], Alu.mult)
                    nc.vector.tensor_scalar(t2[:], t2[:], -0.5, 1.5, Alu.mult,
                                            Alu.add)
                    nc.vector.tensor_tensor(y[:], y[:], t2[:], Alu.mult)
                a_s = fp.tile([F, 1], f32)
                nc.vector.tensor_tensor(a_s[:], y[:], gam[:], Alu.mult)
                b_aff = fp.tile([F, 1], f32)
                nc.vector.tensor_tensor(b_aff[:], mean[:], a_s[:], Alu.mult)
                nc.vector.tensor_tensor(b_aff[:], bet[:], b_aff[:],
                                        Alu.subtract)
                b16 = fp.tile([F, 1], f16)
                nc.vector.tensor_copy(b16[:], b_aff[:])
                pbf = fps.tile([2 * H, 1], f32, tag="pbf")
                nc.tensor.matmul(pbf[:], wzx[:], b16[:],
                                 start=True, stop=True)
                nc.vector.tensor_tensor(zb_f[:], bz[:], pbf[:], Alu.add)
                # scaled x-weights at partitions 64..66 (chain A, q64) and
                # 96..98 (chain B, q96); bases must match the gx rows
                a67 = fp.tile([96 + F, 1], f32)
                nc.sync.dma_start(a67[H:H + F, :], a_s[:])
                nc.sync.dma_start(a67[96:96 + F, :], a_s[:])
                nc.sync.dma_start(wzx6[H:H + F, :], wzxb_d[:])
                nc.sync.dma_start(wzx6[96:96 + F, :], wzxb_d[:])
                nc.vector.tensor_scalar(wzx6[H:H + F, :], wzx6[H:H + F, :],
                                        a67[H:H + F, 0:1], None, Alu.mult)
                nc.vector.tensor_scalar(wzx6[96:96 + F, :],
                                        wzx6[96:96 + F, :],
                                        a67[96:96 + F, 0:1], None, Alu.mult)

            # ---- phase 4: the scan ---------------------------------------
            # State between steps is the UNSUMMED blend pair rz=[u1;u2]
            # ([128,B]); wzdup's duplicated rows compute Wh@(u1+u2). The gx
            # stream tile holds the Chebyshev rows T_0..T_10 at partitions
            # 0-10 and x_t at 64-66. NG independent 512-col batch chains
            # interleave on the engines; pz/pg are full-width 2-bank PSUM
            # tiles whose column halves belong to the chains (subtile deps
            # keep the chains independent).
            with tc.tile_pool(name="rz", bufs=4) as rzp, \
                 tc.tile_pool(name="gx", bufs=8) as gxp, \
                 tc.tile_pool(name="tg", bufs=4) as tgp, \
                 tc.tile_pool(name="ps_z", bufs=2, space="PSUM") as pzp, \
                 tc.tile_pool(name="ps_g", bufs=2, space="PSUM") as pgp:
                rzs = []
                for c in range(NG):
                    rz = rzp.tile([2 * H, 512], f16, tag=f"rz{c}")
                    nc.vector.memset(rz[:], 0.0)
                    rzs.append(rz)

                def gx_tile(s):
                    gx = gxp.tile([96 + F, 512], bf16, tag="gx")
                    # four transfers on two trigger queues (sync + scalar) so
                    # the DMA work for one step executes two-way parallel
                    nc.sync.dma_start(
                        gx[0:NK, :],
                        chd[s:s + 1, 0:NK * 512].rearrange(
                            "a (p n) -> (a p) n", p=NK))
                    nc.gpsimd.dma_start(
                        gx[32:32 + NK, :],
                        chd[s:s + 1, NK * 512:2 * NK * 512].rearrange(
                            "a (p n) -> (a p) n", p=NK))
                    nc.sync.dma_start(gx[H:H + F, :],
                                      xT[F * s:F * s + F, 0:512])
                    nc.gpsimd.dma_start(gx[96:96 + F, :],
                                         xT[F * s:F * s + F, 512:b_loc])
                    return gx

                def stream_mms(pg_t, pz_t, gx_t):
                    # four stream matmuls on four distinct PE row groups ->
                    # they execute concurrently in the 32-row sub-arrays
                    nc.tensor.matmul(pz_t[:, 0:512], wzx6[H:H + F, :],
                                     gx_t[H:H + F, :],
                                     start=True, stop=False)
                    nc.tensor.matmul(pz_t[:, 512:b_loc], wzx6[96:96 + F, :],
                                     gx_t[96:96 + F, :],
                                     start=True, stop=False,
                                     tile_position=(96, 0))
                    nc.tensor.matmul(pg_t[:, 0:512], gcoef[0:NK, :],
                                     gx_t[0:NK, :], start=True, stop=True)
                    nc.tensor.matmul(pg_t[:, 512:b_loc], gcoef[32:32 + NK, :],
                                     gx_t[32:32 + NK, :],
                                     start=True, stop=True)

                gxs = [gx_tile(i) for i in range(5)]
                # prologue: step 0's gate matmuls and the opening
                # x-contribution (the main matmul is the closing accumulant)
                pg_cur = pgp.tile([2 * H, b_loc], f32, tag="pg")
                pz_cur = pzp.tile([2 * H, b_loc], f32, tag="pz")
                stream_mms(pg_cur, pz_cur, gxs[0])

                for s in range(s_steps):
                    if s % 2 == 0:
                        next(ph3_rest, None)
                    gx_cur = gxs[0]
                    gxs = gxs[1:] + ([gx_tile(s + 5)]
                                     if s + 5 < s_steps else [])
                    # chain A's main first; the stream quad for s+1 goes
                    # between the chains - by the time the PE reaches it,
                    # tanhB(s-1) and blendB(s-1) have released its buffers so
                    # all four small-K matmuls run concurrently on their four
                    # row groups, leaving nothing between blendA(s) and
                    # mainA(s+1) but one LDWEIGHTS
                    pg_nxt = pz_nxt = None
                    if s + 1 < s_steps:
                        pg_nxt = pgp.tile([2 * H, b_loc], f32, tag="pg",
                                          name="pgn")
                        pz_nxt = pzp.tile([2 * H, b_loc], f32, tag="pz",
                                          name="pzn")
                    for c in range(NG):
                        cs = slice(c * 512, (c + 1) * 512)
                        nc.tensor.matmul(pz_cur[:, cs], wzdup[:], rzs[c][:],
                                         start=False, stop=True)
                        T = tgp.tile([2 * H, 512], f16, tag=f"T{c}")
                        nc.scalar.activation(T[:], pz_cur[:, cs], Act.Tanh,
                                             bias=zb_f[:])
                        rz_n = rzp.tile([2 * H, 512], f16, tag=f"rz{c}")
                        nc.vector.tensor_tensor(rz_n[:], T[:], pg_cur[:, cs],
                                                Alu.mult)
                        rzs[c] = rz_n
                        if c == 0 and pg_nxt is not None:
                            stream_mms(pg_nxt, pz_nxt, gxs[0])
                    pg_cur, pz_cur = pg_nxt, pz_nxt

                # ---- phase 5: classifier (per chain) ---------------------
                for c in range(NG):
                    cs = slice(c * 512, (c + 1) * 512)
                    ph = pzp.tile([H, 512], f32, tag="pz")
                    nc.tensor.matmul(ph[:], idup[:], rzs[c][:],
                                     start=True, stop=True)
                    hf = tgp.tile([H, 512], f16, tag=f"T{c}")
                    nc.scalar.copy(hf[:], ph[:])
                    pcls = pgp.tile([2, 512], f32, tag="pg")
                    nc.tensor.matmul(pcls[:], wcls[:], hf[:],
                                     start=True, stop=True)
                    pr = tgp.tile([2, 512], f32, tag=f"pr{c}")
                    nc.scalar.activation(pr[:], pcls[:], Act.Sigmoid,
                                         bias=bcls[:])
                    nc.sync.dma_start(probs_d[:, cs], pr[:])

            cgk.release()
            cgp.release()

    nc.compile()
    return nc


def _host_prep(inputs, num_cores, s_steps, b_loc):
    """Layout-only host prep: shard batch, transpose to feature-major,
    stack/transpose weights plus input-independent constants (Chebyshev
    sampling grid and LSQ projection matrix). No input-data arithmetic."""
    x = np.asarray(inputs["x"], dtype=np.float32)
    times = np.asarray(inputs["times"], dtype=np.float32)
    ff1_w = np.asarray(inputs["ff1_w"], np.float32)
    ff2_w = np.asarray(inputs["ff2_w"], np.float32)
    ff1_b = np.asarray(inputs["ff1_b"], np.float32)
    ff2_b = np.asarray(inputs["ff2_b"], np.float32)
    ta_w = np.asarray(inputs["ta_w"], np.float32)
    ta_b = np.asarray(inputs["ta_b"], np.float32)
    tb_w = np.asarray(inputs["tb_w"], np.float32)
    tb_b = np.asarray(inputs["tb_b"], np.float32)
    cls_w = np.asarray(inputs["cls_w"], np.float32)
    cls_b = np.asarray(inputs["cls_b"], np.float32)
    gam = np.asarray(inputs["bn_gamma"], np.float32)
    bet = np.asarray(inputs["bn_beta"], np.float32)

    Wst = np.concatenate([ff1_w, ff2_w], 0)  # [128, 67]
    whT = np.ascontiguousarray(Wst[:, F:].T)       # [64, 128]
    wzdup = np.concatenate([whT, whT], 0).astype(np.float16)
    wzx = np.ascontiguousarray(Wst[:, :F].T).astype(np.float16)  # [3, 128]
    wzxb = np.ascontiguousarray(Wst[:, :F].T).astype(ml_dtypes.bfloat16)
    idup = np.concatenate([np.eye(H), np.eye(H)], 0).astype(np.float16)
    bz = np.concatenate([ff1_b, ff2_b]).reshape(2 * H, 1).astype(np.float32)
    A = ta_w[:, 0]
    Bc = ta_b + tb_w[:, 0]
    Cc = tb_b
    # gate pair coefficients: columns 0-63 give +z (g), 64-127 give -z (1-g)
    wg3 = np.stack([np.concatenate([A, -A]),
                    np.concatenate([Bc, -Bc]),
                    np.concatenate([Cc, -Cc])], 0).astype(np.float16)
    # constant Chebyshev sampling nodes on [0,10] and LSQ projection
    theta = (np.arange(MFIT) + 0.5) * np.pi / MFIT
    cg = 5.0 + 5.0 * np.cos(theta)
    gridpow = np.stack([cg * cg, cg, np.ones_like(cg)], 0).astype(np.float16)
    cp = (cg - 5.0) / 5.0
    V = np.empty((MFIT, NK), np.float64)
    V[:, 0] = 1.0
    V[:, 1] = cp
    for k in range(2, NK):
        V[:, k] = 2 * cp * V[:, k - 1] - V[:, k - 2]
    Wfit = np.linalg.solve(V.T @ V + 1e-6 * np.eye(NK), V.T).T  # [MFIT, NK]
    wfit44 = np.empty((MFIT // 4, 4 * NK), np.float32)
    for m in range(4):
        wfit44[:, m * NK:(m + 1) * NK] = Wfit[m * 128:(m + 1) * 128, :]
    wfit44 = wfit44.astype(np.float16)

    d0 = cls_w[0] - cls_w[1]
    wcls = np.stack([d0, -d0], 1).astype(np.float16)  # [64, 2]
    bcls = np.array([cls_b[0] - cls_b[1],
                     cls_b[1] - cls_b[0]]).reshape(2, 1).astype(np.float32)
    k = np.arange(128)
    eT = np.concatenate(
        [((p + k[:, None]) % 3 == np.arange(3)[None, :]) for p in range(3)],
        axis=1).astype(ml_dtypes.bfloat16)  # [128, 9]

    shared = dict(
        wzdup=wzdup, wzx=wzx, wzxb=wzxb, idup=idup, bz=bz, wg3=wg3,
        gridpow=gridpow, wfit44=wfit44, eT=eT,
        gam=gam.reshape(F, 1).astype(np.float32),
        bet=bet.reshape(F, 1).astype(np.float32),
        wcls=wcls, bcls=bcls)

    in_maps = []
    for c in range(num_cores):
        sl = slice(c * b_loc, (c + 1) * b_loc)
        xc = x[sl, :s_steps, :]                       # [b_loc, S, 3]
        xT = np.ascontiguousarray(xc.transpose(1, 2, 0)).reshape(
            s_steps * F, b_loc).astype(ml_dtypes.bfloat16)
        tTc = np.ascontiguousarray(times[sl, :s_steps, 0].T).astype(np.float32)
        in_maps.append(dict(shared, xT=xT, tT=tTc))
    return in_maps


def kernel(**inputs):
    import time
    from concourse.bass_utils import run_bass_kernel_spmd

    num_cores, s_steps, b_loc = NUM_CORES, S_FULL, B_FULL // NUM_CORES
    key = (num_cores, s_steps, b_loc)
    if key not in _CACHE:
        _CACHE[key] = _build(*key)
    nc = _CACHE[key]
    in_maps = _host_prep(inputs, num_cores, s_steps, b_loc)
    res = None
    for attempt in range(3):
        try:
            res = run_bass_kernel_spmd(nc, in_maps,
                                       core_ids=list(range(num_cores)))
            break
        except Exception:
            if attempt == 2:
                raise
            time.sleep(5.0)  # transient NRT exec-unit errors recover on retry
    out = np.empty((num_cores * b_loc, 2), np.float32)
    for c in range(num_cores):
        out[c * b_loc:(c + 1) * b_loc] = res.results[c]["probs"].T
    return out


# revision 23
# speedup vs baseline: 1.0932x; 1.0932x over previous
# Trainium2 Bass kernel for nn_LiquidMalwareDetector.
#
# Strategy: pure data parallelism over the batch dim (8192 -> 1024 per core,
# 8 cores). Host side only reshapes/shards (layout, no arithmetic); all math
# runs on device:
#   phase 1: per-core BN channel sums/sumsq (matmul-with-0/1-matrix reduction)
#            + 8-core AllReduce of the 6 partial stats
#   phase 2b: fit a degree-10 Chebyshev polynomial (in clipped time c) to the
#            128 gate curves g_h(c)=sigmoid((A_h c + B_h) c + C_h) ON DEVICE:
#            sample sigmoid at 512 Chebyshev nodes (PE matmul + ACT sigmoid),
#            then project onto the basis with a constant pinv matrix (PE).
#            This removes the sigmoid ACT op from the scan entirely - the
#            per-step gate becomes a K=11 PE matmul against the coefficients.
#   phase 2: fold BN affine into the x-columns of the ff weights + bias
#   phase 3: stage the bf16 Chebyshev basis rows T_0..T_10(c') for every step
#            to DRAM (DVE recurrence over 128-step tiles; overlaps the scan)
#   phase 4: 1024-step recurrence in feature-major layout, two independent
#            512-column batch chains interleaved to hide the serial latency.
#            State is the unsummed blend pair rz=[g*t1; (1-g)*t2]; duplicated
#            Wh rows in the K=128 matmul sum the pair. Per-iteration PE issue
#            order is [gateA', xA', mainA, gateB', xB', mainB] so ready work
#            sits ahead of each blocking main matmul. The x-contribution for
#            step s+1 is pre-accumulated into the next (2-bank wide) PSUM
#            tile so tanh starts right after the main matmul drains; gate and
#            x run as single N=1024 bf16 streams on disjoint PE row groups
#            (q0/q64), concurrently. The blend multiplies tanh output (SBUF
#            f16) with the gate (PSUM f32) in one DVE op.
#   phase 5: classifier via sigmoid of logit-difference (== 2-class softmax)
import numpy as np
import ml_dtypes

NUM_CORES = 8
B_FULL = 8192
S_FULL = 1024
F = 3
H = 64
BN_EPS = 1e-5
DEG = 10            # Chebyshev degree for the gate fit
NK = DEG + 1        # basis rows
MFIT = 512          # sampling nodes for the on-device fit

_CACHE = {}


def _build(num_cores, s_steps, b_loc):
    from concourse import bacc, mybir
    import concourse.tile as tile

    f16 = mybir.dt.float16
    bf16 = mybir.dt.bfloat16
    f32 = mybir.dt.float32
    Alu = mybir.AluOpType
    Act = mybir.ActivationFunctionType

    NG = b_loc // 512  # 512-column groups per step
    assert b_loc % 512 == 0

    nc = bacc.Bacc(
        "TRN2",
        target_bir_lowering=False,
        debug=False,
        num_devices=num_cores,
    )

    # ---- I/O -------------------------------------------------------------
    xT = nc.dram_tensor("xT", [s_steps * F, b_loc], bf16,
                        kind="ExternalInput")
    tT = nc.dram_tensor("tT", [s_steps, b_loc], f32, kind="ExternalInput")
    # wzdup: [128,128] duplicated h-weights (rows 0-63 == rows 64-127 == Wh.T)
    # so the matmul itself sums the unsummed blend halves [u1; u2].
    wzdup_d = nc.dram_tensor("wzdup", [2 * H, 2 * H], f16, kind="ExternalInput")
    wzx_d = nc.dram_tensor("wzx", [F, 2 * H], f16, kind="ExternalInput")
    wzxb_d = nc.dram_tensor("wzxb", [F, 2 * H], bf16, kind="ExternalInput")
    idup_d = nc.dram_tensor("idup", [2 * H, H], f16, kind="ExternalInput")
    bz_d = nc.dram_tensor("bz", [2 * H, 1], f32, kind="ExternalInput")
    # gate fit inputs: wg3 rows = [A;B;C] per h-pair column (+/- stacked),
    # gridpow rows = [c^2; c; 1] at the sampling nodes, wfit44 = the constant
    # LSQ projection matrix chunked [128, 4*11].
    wg3_d = nc.dram_tensor("wg3", [3, 2 * H], f16, kind="ExternalInput")
    gridpow_d = nc.dram_tensor("gridpow", [3, MFIT], f16, kind="ExternalInput")
    wfit_d = nc.dram_tensor("wfit44", [MFIT // 4, 4 * NK], f16,
                            kind="ExternalInput")
    eT_d = nc.dram_tensor("eT", [128, 9], bf16, kind="ExternalInput")
    gam_d = nc.dram_tensor("gam", [F, 1], f32, kind="ExternalInput")
    bet_d = nc.dram_tensor("bet", [F, 1], f32, kind="ExternalInput")
    wcls_d = nc.dram_tensor("wcls", [H, 2], f16, kind="ExternalInput")
    bcls_d = nc.dram_tensor("bcls", [2, 1], f32, kind="ExternalInput")
    probs_d = nc.dram_tensor("probs", [2, b_loc], f32, kind="ExternalOutput")

    inv_bs = 1.0 / float(num_cores * b_loc * s_steps)

    with tile.TileContext(nc) as tc:
        with tc.tile_pool(name="const", bufs=1) as cpool, \
             tc.tile_pool(name="dram", bufs=1, space="DRAM") as dpool:
            # persistent SBUF-resident weights/constants
            wzdup = cpool.tile([2 * H, 2 * H], f16)
            nc.sync.dma_start(wzdup[:], wzdup_d[:])
            wzx = cpool.tile([F, 2 * H], f16)
            nc.sync.dma_start(wzx[:], wzx_d[:])
            # folded x-weights, placed at partitions 64..66 so the x matmul's
            # lhsT/rhs base partitions match (row group q64, disjoint from the
            # gate matmul's q0 rows -> the two run concurrently on the PE)
            wzx6 = cpool.tile([96 + F, 2 * H], bf16)
            idup = cpool.tile([2 * H, H], f16)
            nc.sync.dma_start(idup[:], idup_d[:])
            bz = cpool.tile([2 * H, 1], f32)
            nc.sync.dma_start(bz[:], bz_d[:])
            wg3 = cpool.tile([3, 2 * H], f16)
            nc.sync.dma_start(wg3[:], wg3_d[:])
            gridpow = cpool.tile([3, MFIT], f16)
            nc.sync.dma_start(gridpow[:], gridpow_d[:])
            wfit = cpool.tile([MFIT // 4, 4 * NK], f16)
            nc.sync.dma_start(wfit[:], wfit_d[:])
            eT = cpool.tile([128, 9], bf16)
            nc.sync.dma_start(eT[:], eT_d[:])
            gam = cpool.tile([F, 1], f32)
            nc.sync.dma_start(gam[:], gam_d[:])
            bet = cpool.tile([F, 1], f32)
            nc.sync.dma_start(bet[:], bet_d[:])
            wcls = cpool.tile([H, 2], f16)
            nc.sync.dma_start(wcls[:], wcls_d[:])
            bcls = cpool.tile([2, 1], f32)
            nc.sync.dma_start(bcls[:], bcls_d[:])
            zb_f = cpool.tile([2 * H, 1], f32)   # folded tanh bias
            stats_g = cpool.tile([F, 2], f32)    # all-reduced [sum, sumsq]
            gcoef = cpool.tile([32 + NK, 2 * H], bf16)  # gate coefficients
            # (rows 0-10 serve chain A at row group q0, a DMA-duplicated copy
            # at rows 32-42 serves chain B at q32 so all four per-step stream
            # matmuls land on distinct PE row groups and run concurrently)
            ones = cpool.tile([128, b_loc], bf16)  # T_0 plane
            nc.vector.memset(ones[:], 1.0)

            # ---- phase 3: Chebyshev basis staging ------------------------
            # chd row s holds the A-half block [T_0..T_10] then the B-half
            # block, each 512 wide, so the scan reads two contiguous 11KB
            # blocks per step. Tile 0 (steps 0-127) is built before the scan;
            # tiles 1..7 are interleaved into the scan loop one DVE op per
            # step so the scan's blends are not queued behind ~110us of
            # serial basis work on the Vector engine.
            chd = dpool.tile([s_steps, 2 * NK * 512], bf16)
            cgp = tc.alloc_tile_pool(name="cg", bufs=2)
            cgk = tc.alloc_tile_pool(name="cgk", bufs=4)

            def ph3_tile(i):
                r0 = i * 128
                nr = min(128, s_steps - r0)
                tt = cgp.tile([128, b_loc], f32, tag="tt", name="tt")
                nc.gpsimd.dma_start(tt[0:nr, :], tT[r0:r0 + nr, :])
                cc = cgp.tile([128, b_loc], f32, tag="cc", name="cc")
                nc.vector.tensor_scalar(cc[0:nr, :], tt[0:nr, :], 60.0,
                                        0.0, Alu.min, Alu.max)
                nc.gpsimd.dma_start(chd[r0:r0 + nr, 0:512],
                                    ones[0:nr, 0:512])
                nc.gpsimd.dma_start(
                    chd[r0:r0 + nr, NK * 512:NK * 512 + 512],
                    ones[0:nr, 512:b_loc])
                yield
                t1c = cgk.tile([128, b_loc], bf16, tag="tk", name="t1c")
                nc.vector.tensor_scalar(t1c[0:nr, :], cc[0:nr, :], 0.2,
                                        -1.0, Alu.mult, Alu.add)
                nc.gpsimd.dma_start(chd[r0:r0 + nr, 512:1024],
                                    t1c[0:nr, 0:512])
                nc.gpsimd.dma_start(
                    chd[r0:r0 + nr, (NK + 1) * 512:(NK + 2) * 512],
                    t1c[0:nr, 512:b_loc])
                yield
                dd = cgp.tile([128, b_loc], bf16, tag="dd", name="dd")
                nc.vector.tensor_scalar(dd[0:nr, :], cc[0:nr, :], 0.4,
                                        -2.0, Alu.mult, Alu.add)
                yield
                prev2, prev1 = ones, t1c
                for k in range(2, NK):
                    tmp = cgp.tile([128, b_loc], bf16, tag="tmp", name="tmp")
                    nc.vector.tensor_tensor(tmp[0:nr, :], dd[0:nr, :],
                                            prev1[0:nr, :], Alu.mult)
                    yield
                    tk = cgk.tile([128, b_loc], bf16, tag="tk", name="tk")
                    nc.vector.tensor_tensor(tk[0:nr, :], tmp[0:nr, :],
                                            prev2[0:nr, :], Alu.subtract)
                    nc.gpsimd.dma_start(
                        chd[r0:r0 + nr, k * 512:(k + 1) * 512],
                        tk[0:nr, 0:512])
                    nc.gpsimd.dma_start(
                        chd[r0:r0 + nr, (NK + k) * 512:(NK + k + 1) * 512],
                        tk[0:nr, 512:b_loc])
                    yield
                    prev2, prev1 = prev1, tk

            import itertools
            ph3_rest = itertools.chain(
                *[ph3_tile(i) for i in range(1, (s_steps + 127) // 128)])

            # ---- phase 1: BN stats ---------------------------------------
            rows_total = s_steps * F
            with tc.tile_pool(name="st_ps", bufs=1, space="PSUM") as stps, \
                 tc.tile_pool(name="st_sb", bufs=3) as stsb:
                psum_s = stps.tile([F, b_loc], f32)
                psum_q = stps.tile([F, b_loc], f32)
                n_tiles = (rows_total + 127) // 128
                for i in range(n_tiles):
                    r0 = i * 128
                    nr = min(128, rows_total - r0)
                    ph = r0 % 3
                    xst = stsb.tile([128, b_loc], bf16, tag="xst")
                    nc.sync.dma_start(xst[0:nr, :], xT[r0:r0 + nr, :])
                    xsq = stsb.tile([128, b_loc], bf16, tag="xsq")
                    nc.vector.tensor_tensor(
                        xsq[0:nr, :], xst[0:nr, :], xst[0:nr, :], Alu.mult)
                    first = i == 0
                    last = i == n_tiles - 1
                    for g in range(NG):
                        cs = slice(g * 512, (g + 1) * 512)
                        nc.tensor.matmul(
                            psum_s[:, cs], eT[0:nr, 3 * ph:3 * ph + 3],
                            xst[0:nr, cs], start=first, stop=last)
                        nc.tensor.matmul(
                            psum_q[:, cs], eT[0:nr, 3 * ph:3 * ph + 3],
                            xsq[0:nr, cs], start=first, stop=last)
                stats_l = stsb.tile([F, 2], f32, tag="stl")
                nc.vector.tensor_reduce(
                    stats_l[:, 0:1], psum_s[:], mybir.AxisListType.X, Alu.add)
                nc.vector.tensor_reduce(
                    stats_l[:, 1:2], psum_q[:], mybir.AxisListType.X, Alu.add)
                # 6-float AllReduce across the 8 cores via DRAM bounce
                cc_in = dpool.tile([F, 2], f32)
                cc_out = dpool.tile([F, 2], f32, addr_space="Shared")
                nc.sync.dma_start(cc_in[:], stats_l[:])
                nc.gpsimd.collective_compute(
                    "AllReduce", Alu.add,
                    replica_groups=[list(range(num_cores))],
                    ins=[cc_in.opt()], outs=[cc_out.opt()])
                nc.sync.dma_start(stats_g[:], cc_out[:])

            # ---- phase 2b: on-device gate fit + HAM bridge + BN fold -----
            with tc.tile_pool(name="fold", bufs=1) as fp, \
                 tc.tile_pool(name="gfit", bufs=2) as gf, \
                 tc.tile_pool(name="fold_ps", bufs=2, space="PSUM") as fps, \
                 tc.tile_pool(name="coef_ps", bufs=1, space="PSUM") as cps:
                # sample the 128 gate sigmoid curves at MFIT Chebyshev nodes
                # and project onto the degree-DEG basis (all on PE/ACT).
                pcoef = cps.tile([NK, 2 * H], f32)
                for m in range(4):
                    pgs = fps.tile([128, 128], f32, tag="pgs")
                    nc.tensor.matmul(
                        pgs[:], gridpow[:, m * 128:(m + 1) * 128], wg3[:],
                        start=True, stop=True)
                    gsm = gf.tile([128, 128], f16, tag="gsm")
                    nc.scalar.activation(gsm[:], pgs[:], Act.Sigmoid)
                    nc.tensor.matmul(
                        pcoef[:], wfit[:, m * NK:(m + 1) * NK], gsm[:],
                        start=(m == 0), stop=(m == 3))
                nc.vector.tensor_copy(gcoef[0:NK, :], pcoef[:])
                nc.sync.dma_start(gcoef[32:32 + NK, :], gcoef[0:NK, :])

                # ---- phase 2: BN fold ------------------------------------
                mean = fp.tile([F, 1], f32)
                nc.vector.tensor_scalar(mean[:], stats_g[:, 0:1], inv_bs, None,
                                        Alu.mult)
                msq = fp.tile([F, 1], f32)
                nc.vector.tensor_scalar(msq[:], stats_g[:, 1:2], inv_bs, None,
                                        Alu.mult)
                var = fp.tile([F, 1], f32)
                nc.vector.tensor_tensor(var[:], mean[:], mean[:], Alu.mult)
                nc.vector.tensor_tensor(var[:], msq[:], var[:], Alu.subtract)
                veps = fp.tile([F, 1], f32)
                nc.vector.tensor_scalar(veps[:], var[:], BN_EPS, None, Alu.add)
                # rsqrt: ACT sqrt seed + DVE reciprocal + 2 Newton iters
                sq = fp.tile([F, 1], f32)
                nc.scalar.activation(sq[:], veps[:], Act.Sqrt)
                y = fp.tile([F, 1], f32)
                nc.vector.reciprocal(y[:], sq[:])
                t1 = fp.tile([F, 1], f32)
                t2 = fp.tile([F, 1], f32)
                for _ in range(2):
                    nc.vector.tensor_tensor(t1[:], y[:], y[:], Alu.mult)
                    nc.vector.tensor_tensor(t2[:], t1[:], veps[:], Alu.mult)
                    nc.vector.tensor_scalar(t2[:], t2[:], -0.5, 1.5, Alu.mult,
                                            Alu.add)
                    nc.vector.tensor_tensor(y[:], y[:], t2[:], Alu.mult)
                a_s = fp.tile([F, 1], f32)
                nc.vector.tensor_tensor(a_s[:], y[:], gam[:], Alu.mult)
                b_aff = fp.tile([F, 1], f32)
                nc.vector.tensor_tensor(b_aff[:], mean[:], a_s[:], Alu.mult)
                nc.vector.tensor_tensor(b_aff[:], bet[:], b_aff[:],
                                        Alu.subtract)
                b16 = fp.tile([F, 1], f16)
                nc.vector.tensor_copy(b16[:], b_aff[:])
                pbf = fps.tile([2 * H, 1], f32, tag="pbf")
                nc.tensor.matmul(pbf[:], wzx[:], b16[:],
                                 start=True, stop=True)
                nc.vector.tensor_tensor(zb_f[:], bz[:], pbf[:], Alu.add)
                # scaled x-weights at partitions 64..66 (chain A, q64) and
                # 96..98 (chain B, q96); bases must match the gx rows
                a67 = fp.tile([96 + F, 1], f32)
                nc.sync.dma_start(a67[H:H + F, :], a_s[:])
                nc.sync.dma_start(a67[96:96 + F, :], a_s[:])
                nc.sync.dma_start(wzx6[H:H + F, :], wzxb_d[:])
                nc.sync.dma_start(wzx6[96:96 + F, :], wzxb_d[:])
                nc.vector.tensor_scalar(wzx6[H:H + F, :], wzx6[H:H + F, :],
                                        a67[H:H + F, 0:1], None, Alu.mult)
                nc.vector.tensor_scalar(wzx6[96:96 + F, :],
                                        wzx6[96:96 + F, :],
                                        a67[96:96 + F, 0:1], None, Alu.mult)

            for _ in ph3_tile(0):
                pass

            # ---- phase 4: the scan ---------------------------------------
            # State between steps is the UNSUMMED blend pair rz=[u1;u2]
            # ([128,B]); wzdup's duplicated rows compute Wh@(u1+u2). The gx
            # stream tile holds the Chebyshev rows T_0..T_10 at partitions
            # 0-10 and x_t at 64-66. NG independent 512-col batch chains
            # interleave on the engines; pz/pg are full-width 2-bank PSUM
            # tiles whose column halves belong to the chains (subtile deps
            # keep the chains independent).
            with tc.tile_pool(name="rz", bufs=4) as rzp, \
                 tc.tile_pool(name="gx", bufs=8) as gxp, \
                 tc.tile_pool(name="tg", bufs=4) as tgp, \
                 tc.tile_pool(name="ps_z", bufs=2, space="PSUM") as pzp, \
                 tc.tile_pool(name="ps_g", bufs=2, space="PSUM") as pgp:
                rzs = []
                for c in range(NG):
                    rz = rzp.tile([2 * H, 512], f16, tag=f"rz{c}")
                    nc.vector.memset(rz[:], 0.0)
                    rzs.append(rz)

                def gx_tile(s):
                    gx = gxp.tile([96 + F, 512], bf16, tag="gx")
                    # four transfers on two trigger queues (sync + scalar) so
                    # the DMA work for one step executes two-way parallel
                    nc.sync.dma_start(
                        gx[0:NK, :],
                        chd[s:s + 1, 0:NK * 512].rearrange(
                            "a (p n) -> (a p) n", p=NK))
                    nc.gpsimd.dma_start(
                        gx[32:32 + NK, :],
                        chd[s:s + 1, NK * 512:2 * NK * 512].rearrange(
                            "a (p n) -> (a p) n", p=NK))
                    nc.sync.dma_start(gx[H:H + F, :],
                                      xT[F * s:F * s + F, 0:512])
                    nc.gpsimd.dma_start(gx[96:96 + F, :],
                                         xT[F * s:F * s + F, 512:b_loc])
                    return gx

                def stream_mms(pg_t, pz_t, gx_t):
                    # four stream matmuls on four distinct PE row groups ->
                    # they execute concurrently in the 32-row sub-arrays
                    nc.tensor.matmul(pz_t[:, 0:512], wzx6[H:H + F, :],
                                     gx_t[H:H + F, :],
                                     start=True, stop=False)
                    nc.tensor.matmul(pz_t[:, 512:b_loc], wzx6[96:96 + F, :],
                                     gx_t[96:96 + F, :],
                                     start=True, stop=False,
                                     tile_position=(96, 0))
                    nc.tensor.matmul(pg_t[:, 0:512], gcoef[0:NK, :],
                                     gx_t[0:NK, :], start=True, stop=True)
                    nc.tensor.matmul(pg_t[:, 512:b_loc], gcoef[32:32 + NK, :],
                                     gx_t[32:32 + NK, :],
                                     start=True, stop=True)

                gxs = [gx_tile(i) for i in range(5)]
                # prologue: step 0's gate matmuls and the opening
                # x-contribution (the main matmul is the closing accumulant)
                pg_cur = pgp.tile([2 * H, b_loc], f32, tag="pg")
                pz_cur = pzp.tile([2 * H, b_loc], f32, tag="pz")
                stream_mms(pg_cur, pz_cur, gxs[0])

                for s in range(s_steps):
                    if s % 2 == 0:
                        next(ph3_rest, None)
                    gx_cur = gxs[0]
                    gxs = gxs[1:] + ([gx_tile(s + 5)]
                                     if s + 5 < s_steps else [])
                    # mains first; the stream quad for step s+1 follows
                    # and fills the tanh/blend window of this step
                    for c in range(NG):
                        cs = slice(c * 512, (c + 1) * 512)
                        nc.tensor.matmul(pz_cur[:, cs], wzdup[:], rzs[c][:],
                                         start=False, stop=True)
                        T = tgp.tile([2 * H, 512], f16, tag=f"T{c}")
                        nc.scalar.activation(T[:], pz_cur[:, cs], Act.Tanh,
                                             bias=zb_f[:])
                        rz_n = rzp.tile([2 * H, 512], f16, tag=f"rz{c}")
                        nc.vector.tensor_tensor(rz_n[:], T[:], pg_cur[:, cs],
                                                Alu.mult)
                        rzs[c] = rz_n
                    pg_nxt = pz_nxt = None
                    if s + 1 < s_steps:
                        pg_nxt = pgp.tile([2 * H, b_loc], f32, tag="pg",
                                          name="pgn")
                        pz_nxt = pzp.tile([2 * H, b_loc], f32, tag="pz",
                                          name="pzn")
                        stream_mms(pg_nxt, pz_nxt, gxs[0])
                    pg_cur, pz_cur = pg_nxt, pz_nxt

                # ---- phase 5: classifier (per chain) ---------------------
                for c in range(NG):
                    cs = slice(c * 512, (c + 1) * 512)
                    ph = pzp.tile([H, 512], f32, tag="pz")
                    nc.tensor.matmul(ph[:], idup[:], rzs[c][:],
                                     start=True, stop=True)
                    hf = tgp.tile([H, 512], f16, tag=f"T{c}")
                    nc.scalar.copy(hf[:], ph[:])
                    pcls = pgp.tile([2, 512], f32, tag="pg")
                    nc.tensor.matmul(pcls[:], wcls[:], hf[:],
                                     start=True, stop=True)
                    pr = tgp.tile([2, 512], f32, tag=f"pr{c}")
                    nc.scalar.activation(pr[:], pcls[:], Act.Sigmoid,
                                         bias=bcls[:])
                    nc.sync.dma_start(probs_d[:, cs], pr[:])

            cgk.release()
            cgp.release()

    nc.compile()
    return nc


def _host_prep(inputs, num_cores, s_steps, b_loc):
    """Layout-only host prep: shard batch, transpose to feature-major,
    stack/transpose weights plus input-independent constants (Chebyshev
    sampling grid and LSQ projection matrix). No input-data arithmetic."""
    x = np.asarray(inputs["x"], dtype=np.float32)
    times = np.asarray(inputs["times"], dtype=np.float32)
    ff1_w = np.asarray(inputs["ff1_w"], np.float32)
    ff2_w = np.asarray(inputs["ff2_w"], np.float32)
    ff1_b = np.asarray(inputs["ff1_b"], np.float32)
    ff2_b = np.asarray(inputs["ff2_b"], np.float32)
    ta_w = np.asarray(inputs["ta_w"], np.float32)
    ta_b = np.asarray(inputs["ta_b"], np.float32)
    tb_w = np.asarray(inputs["tb_w"], np.float32)
    tb_b = np.asarray(inputs["tb_b"], np.float32)
    cls_w = np.asarray(inputs["cls_w"], np.float32)
    cls_b = np.asarray(inputs["cls_b"], np.float32)
    gam = np.asarray(inputs["bn_gamma"], np.float32)
    bet = np.asarray(inputs["bn_beta"], np.float32)

    Wst = np.concatenate([ff1_w, ff2_w], 0)  # [128, 67]
    whT = np.ascontiguousarray(Wst[:, F:].T)       # [64, 128]
    wzdup = np.concatenate([whT, whT], 0).astype(np.float16)
    wzx = np.ascontiguousarray(Wst[:, :F].T).astype(np.float16)  # [3, 128]
    wzxb = np.ascontiguousarray(Wst[:, :F].T).astype(ml_dtypes.bfloat16)
    idup = np.concatenate([np.eye(H), np.eye(H)], 0).astype(np.float16)
    bz = np.concatenate([ff1_b, ff2_b]).reshape(2 * H, 1).astype(np.float32)
    A = ta_w[:, 0]
    Bc = ta_b + tb_w[:, 0]
    Cc = tb_b
    # gate pair coefficients: columns 0-63 give +z (g), 64-127 give -z (1-g)
    wg3 = np.stack([np.concatenate([A, -A]),
                    np.concatenate([Bc, -Bc]),
                    np.concatenate([Cc, -Cc])], 0).astype(np.float16)
    # constant Chebyshev sampling nodes on [0,10] and LSQ projection
    theta = (np.arange(MFIT) + 0.5) * np.pi / MFIT
    cg = 5.0 + 5.0 * np.cos(theta)
    gridpow = np.stack([cg * cg, cg, np.ones_like(cg)], 0).astype(np.float16)
    cp = (cg - 5.0) / 5.0
    V = np.empty((MFIT, NK), np.float64)
    V[:, 0] = 1.0
    V[:, 1] = cp
    for k in range(2, NK):
        V[:, k] = 2 * cp * V[:, k - 1] - V[:, k - 2]
    Wfit = np.linalg.solve(V.T @ V + 1e-6 * np.eye(NK), V.T).T  # [MFIT, NK]
    wfit44 = np.empty((MFIT // 4, 4 * NK), np.float32)
    for m in range(4):
        wfit44[:, m * NK:(m + 1) * NK] = Wfit[m * 128:(m + 1) * 128, :]
    wfit44 = wfit44.astype(np.float16)

    d0 = cls_w[0] - cls_w[1]
    wcls = np.stack([d0, -d0], 1).astype(np.float16)  # [64, 2]
    bcls = np.array([cls_b[0] - cls_b[1],
                     cls_b[1] - cls_b[0]]).reshape(2, 1).astype(np.float32)
    k = np.arange(128)
    eT = np.concatenate(
        [((p + k[:, None]) % 3 == np.arange(3)[None, :]) for p in range(3)],
        axis=1).astype(ml_dtypes.bfloat16)  # [128, 9]

    shared = dict(
        wzdup=wzdup, wzx=wzx, wzxb=wzxb, idup=idup, bz=bz, wg3=wg3,
        gridpow=gridpow, wfit44=wfit44, eT=eT,
        gam=gam.reshape(F, 1).astype(np.float32),
        bet=bet.reshape(F, 1).astype(np.float32),
        wcls=wcls, bcls=bcls)

    in_maps = []
    for c in range(num_cores):
        sl = slice(c * b_loc, (c + 1) * b_loc)
        xc = x[sl, :s_steps, :]                       # [b_loc, S, 3]
        xT = np.ascontiguousarray(xc.transpose(1, 2, 0)).reshape(
            s_steps * F, b_loc).astype(ml_dtypes.bfloat16)
        tTc = np.ascontiguousarray(times[sl, :s_steps, 0].T).astype(np.float32)
        in_maps.append(dict(shared, xT=xT, tT=tTc))
    return in_maps


def kernel(**inputs):
    import time
    from concourse.bass_utils import run_bass_kernel_spmd

    num_cores, s_steps, b_loc = NUM_CORES, S_FULL, B_FULL // NUM_CORES
    key = (num_cores, s_steps, b_loc)
    if key not in _CACHE:
        _CACHE[key] = _build(*key)
    nc = _CACHE[key]
    in_maps = _host_prep(inputs, num_cores, s_steps, b_loc)
    res = None
    for attempt in range(3):
        try:
            res = run_bass_kernel_spmd(nc, in_maps,
                                       core_ids=list(range(num_cores)))
            break
        except Exception:
            if attempt == 2:
                raise
            time.sleep(5.0)  # transient NRT exec-unit errors recover on retry
    out = np.empty((num_cores * b_loc, 2), np.float32)
    for c in range(num_cores):
        out[c * b_loc:(c + 1) * b_loc] = res.results[c]["probs"].T
    return out
